# revision 7
# baseline (speedup 1.0000x reference)
"""Trainium2 Bass kernel for nn_BasicAttention (ragged sequence attention).

Reference computation (per batch b, S=1024, D=256):
    vecs   = vec_table[tokens]          [S, D]
    covecs = covec_table[tokens]        [S, D]
    E      = (vecs @ W) @ covecs^T      [S, S]   (masked to valid prefix L_b)
    ak     = softmax(masked colmax(E)); aq = softmax(masked rowmax(E))
    out    = log_softmax(concat(ak@vecs, aq@covecs) @ lin_w^T + lin_b)

Strategy: data-parallel over batch (4 samples per core x 8 cores), samples
sorted by valid length L and distributed round-robin so each per-core
"slot" has a static extent (max L of its rank group, rounded to 128).

v3 design:
  - One fused transposed dma_gather per slot pulls [vecW | covec] rows in
    [d, s] layout (1KB rows); a tiny straight gather pulls per-token
    classifier projections [vec@lw1^T | covec@lw2^T] (32B rows) -- the
    classifier is folded into the embedding tables on the host, removing
    ~3.4MB/core of gather traffic vs gathering raw [vec|covec] rows.
  - PSUM eviction is fused with exp on ACT: e_exp = exp(T*(E - B0)) with
    accum_out giving rowlse (a log-sum-exp softening of the row max; T=64
    makes the LSE-vs-max gap ~ln(1+s)/64 << tolerance). This deletes the
    DVE pairwise-max tree of the previous version.
  - Col max runs on exp values (monotone): DVE running-max chain over
    q-tiles, PE transpose, reduce -> hatC = exp(T*(colmax-B0)).
  - Softmax weights: exp((ln(lse_or_max) + mask)/T) recovers
    exp(colmax-B0) / exp(rowmax~-B0) with masking; Exp/Ln/Copy share one
    ACT table set (natural_log_exp_and_others) so there are no table
    switches until the end.
  - Invalid positions use an all-zero table row, so E=0 there; with >=513
    valid entries the true max is > 0.2 w.h.p., so exp(T*(0-B0)) ~ 4e-34
    never wins the max and contributes ~0 to sums; masks kill the rest.
"""

import numpy as np
import ml_dtypes

import concourse.bass as bass
import concourse.mybir as mybir
import concourse.tile as tile
from concourse import bacc
from concourse.bass_utils import run_bass_kernel_spmd
from concourse.masks import make_identity

# Problem constants (hardcoded per spec)
B = 32
S = 1024
D = 256
N_CLASSES = 5
N_CORES = 8
BPC = B // N_CORES          # batches (slots) per core
NEG = -30000.0              # large-negative mask (exp() underflows to 0)
T = 64.0                    # lse temperature
B0 = 1.2                    # exp range-safety bias: e_exp = exp(T*(E-B0))

BF16 = mybir.dt.bfloat16
F32 = mybir.dt.float32
I16 = mybir.dt.int16

_cache = {}
_last_key = None


def _build_program(U1, slot_lens=(S,) * BPC, stage=99, repeat=1):
    """Per-core Bass program. U1 = compact table rows; slot_lens = static
    per-slot extents (multiples of 128, descending); repeat for benching."""
    import os
    stage = int(os.environ.get("K_STAGE", stage))
    nc = bacc.Bacc("TRN2", num_devices=N_CORES, debug=False)

    NI = int(sum(slot_lens))            # gathered tokens per core
    NQS = [l // 128 for l in slot_lens]  # per-slot q/k tile counts
    OFF = np.cumsum([0] + list(slot_lens))[:-1]      # token offsets
    NOFF = np.cumsum([0] + list(NQS))[:-1]           # mask-col offsets
    NMC = int(sum(NQS))                 # total mask cols (per half)

    # ---- DRAM I/O ----
    vwcv = nc.dram_tensor("vwcv", [U1, 2 * D], BF16, kind="ExternalInput").ap()
    vpc = nc.dram_tensor("vpc", [U1, 128], BF16, kind="ExternalInput").ap()
    idx = nc.dram_tensor("idx", [128, NI // 16], I16,
                         kind="ExternalInput").ap()
    msk = nc.dram_tensor("msk", [128, 2 * NMC], F32, kind="ExternalInput").ap()
    linb = nc.dram_tensor("linb", [1, N_CLASSES], F32,
                          kind="ExternalInput").ap()
    out = nc.dram_tensor("out", [BPC, N_CLASSES], F32,
                         kind="ExternalOutput").ap()

    with tile.TileContext(nc) as tc:
        with (
            tc.tile_pool(name="const", bufs=1) as cpool,
            tc.tile_pool(name="gath", bufs=1) as gpool,
            tc.tile_pool(name="eall", bufs=2) as epool,
            tc.tile_pool(name="accp", bufs=2) as apool,
            tc.tile_pool(name="small", bufs=2) as spool,
            tc.tile_pool(name="ps_e", bufs=2, space="PSUM") as ps_e,
            tc.tile_pool(name="ps_t", bufs=1, space="PSUM") as ps_t,
            tc.tile_pool(name="ps_s", bufs=2, space="PSUM") as ps_s,
        ):
            # ---- constants / staging ----
            idx_t = cpool.tile([128, NI // 16], I16)
            nc.sync.dma_start(idx_t[:], idx)
            msk_t = cpool.tile([128, 2 * NMC], F32)
            nc.sync.dma_start(msk_t[:], msk)
            linb_t = cpool.tile([1, N_CLASSES], F32)
            nc.sync.dma_start(linb_t[:], linb)
            ident = cpool.tile([128, 128], BF16)
            make_identity(nc, ident[:])
            ones_t = cpool.tile([128, 1], F32)
            nc.vector.memset(ones_t[:], 1.0)
            biasn_t = cpool.tile([128, 1], F32)
            nc.vector.memset(biasn_t[:], -T * B0)
            ones_b = cpool.tile([128, 1], BF16)
            nc.vector.memset(ones_b[:], 1.0)

            for _rep in range(repeat):
              # ---- gathers ----
              # transposed [d,s] rows of [vecW | covec]: one gather per slot
              # so slot 0's matmuls start after one small gather
              vwcvTs = []
              for j in range(BPC):
                  KJ = int(slot_lens[j])
                  oj = int(OFF[j])
                  g_j = gpool.tile([128, 4, KJ], BF16, tag=f"vwcvT{j}")
                  nc.gpsimd.dma_gather(
                      out_ap=g_j[:], in_ap=vwcv,
                      idxs_ap=idx_t[:, oj // 16:(oj + KJ) // 16],
                      num_idxs=KJ, num_idxs_reg=KJ, elem_size=2 * D,
                      transpose=True, single_packet=False,
                  )
                  vwcvTs.append(g_j)
              # straight [s, 16] classifier-projection rows
              vpcs = gpool.tile([128, NI // 128, 128], BF16, tag="vpcs")
              nc.gpsimd.dma_gather(
                  out_ap=vpcs[:], in_ap=vpc, idxs_ap=idx_t[:],
                  num_idxs=NI, num_idxs_reg=NI, elem_size=128,
                  transpose=False, single_packet=False,
              )

              if stage == 0:
                  dbg = spool.tile([128, N_CLASSES], F32, tag="dbg")
                  nc.vector.tensor_copy(dbg[:], vpcs[:, 0, 0:N_CLASSES])
                  nc.sync.dma_start(out[:, :], dbg[0:BPC, :])

              # per-slot log_softmax staging (Ln deferred to one final
              # phase so ACT stays on one table set)
              sums_all = spool.tile([1, BPC], F32, tag="lsm")
              tsb_list = []

              for b in range(BPC if stage > 0 else 0):
                  NQ = NQS[b]
                  KK = int(slot_lens[b])        # k extent (== q extent)
                  off = int(OFF[b])
                  noff = int(NOFF[b])
                  nkc = (KK + 511) // 512       # k chunks of <=512
                  vw = vwcvTs[b]
                  # ---- E tiles; exp-evict with rowlse accum; col max ----
                  e_all = epool.tile([128, NQ, KK], BF16, tag="eall")
                  acc = apool.tile([128, KK], BF16, tag="acc")
                  # lse2 cols [0:NQ] = hatC (max_q exp), [NQ:2NQ] = rowlse
                  lse2 = spool.tile([128, 2 * NQ], F32, tag="lse2")
                  for qt in range(NQ):
                      pe = ps_e.tile([128, 1024], F32, tag="pe")
                      for h in range(2):
                          for kt in range(nkc):
                              kw = min(512, KK - kt * 512)
                              nc.tensor.matmul(
                                  pe[:, kt * 512:kt * 512 + kw],
                                  lhsT=vw[:, h, qt * 128:(qt + 1) * 128],
                                  rhs=vw[:, 2 + h, kt * 512:kt * 512 + kw],
                                  start=(h == 0), stop=(h == 1),
                              )
                      nc.scalar.activation(
                          e_all[:, qt, :], pe[:, 0:KK],
                          mybir.ActivationFunctionType.Exp,
                          scale=T, bias=biasn_t[:],
                          accum_out=lse2[:, NQ + qt:NQ + qt + 1])
                      if qt == 0:
                          nc.vector.tensor_copy(acc[:], e_all[:, 0, :])
                      else:
                          nc.vector.tensor_tensor(
                              out=acc[:], in0=acc[:], in1=e_all[:, qt, :],
                              op=mybir.AluOpType.max)

                  if stage == 1:
                      dbg = spool.tile([128, N_CLASSES], F32, tag="dbg")
                      nc.vector.tensor_copy(dbg[:], acc[:, 0:N_CLASSES])
                      nc.sync.dma_start(out[b:b + 1, :], dbg[0:1, :])
                      continue

                  # ---- col max of exp values: PE transpose + reduce ----
                  pt = ps_t.tile([128, 1024], BF16, tag="pt")
                  for g in range(NQ):
                      nc.tensor.transpose(
                          pt[:, g * 128:(g + 1) * 128],
                          acc[:, g * 128:(g + 1) * 128], ident[:])
                  nc.vector.reduce_max(
                      lse2[:, 0:NQ],
                      pt[:, 0:NQ * 128].rearrange("p (g f) -> p g f", g=NQ),
                      axis=mybir.AxisListType.X)

                  if stage == 2:
                      dbg = spool.tile([128, N_CLASSES], F32, tag="dbg")
                      nc.vector.tensor_copy(dbg[:], lse2[:, 0:N_CLASSES])
                      nc.sync.dma_start(out[b:b + 1, :], dbg[0:1, :])
                      continue

                  # ---- softmax numerators: exp((ln(lse2)+msk)/T) ----
                  lnl = spool.tile([128, 2 * NQ], F32, tag="lnl")
                  nc.scalar.activation(lnl[:], lse2[:],
                                       mybir.ActivationFunctionType.Ln)
                  lnm = spool.tile([128, 2 * NQ], F32, tag="lnm")
                  nc.vector.tensor_tensor(
                      out=lnm[:], in0=lnl[:],
                      in1=msk_t[:, 2 * noff:2 * (noff + NQ)],
                      op=mybir.AluOpType.add)
                  au = spool.tile([128, 2 * NQ], BF16, tag="au")
                  nc.scalar.activation(au[:], lnm[:],
                                       mybir.ActivationFunctionType.Exp,
                                       scale=1.0 / T)

                  if stage == 3:
                      dbg = spool.tile([128, N_CLASSES], F32, tag="dbg")
                      nc.vector.tensor_copy(dbg[:], au[:, 0:N_CLASSES])
                      nc.sync.dma_start(out[b:b + 1, :], dbg[0:1, :])
                      continue

                  # ---- denominators ----
                  pden = ps_s.tile([1, 2 * NQ], F32, tag="ps_small")
                  nc.tensor.matmul(pden[:], lhsT=ones_b[:], rhs=au[:],
                                   start=True, stop=True)
                  den = spool.tile([1, 2], F32, tag="den")
                  nc.vector.reduce_sum(
                      den[:], pden[:].rearrange("p (h q) -> p h q", h=2),
                      axis=mybir.AxisListType.X)
                  recip = spool.tile([1, 2], F32, tag="recip")
                  nc.vector.reciprocal(recip[:], den[:])

                  # ---- weighted sums of classifier projections ----
                  psk = ps_s.tile([1, 16], F32, tag="ps_small")
                  psq = ps_s.tile([1, 16], F32, tag="ps_small")
                  goff = off // 128
                  for g in range(NQ):
                      nc.tensor.matmul(
                          psk[:], lhsT=au[:, g:g + 1],
                          rhs=vpcs[:, goff + g, 0:16],
                          start=(g == 0), stop=(g == NQ - 1))
                  for g in range(NQ):
                      nc.tensor.matmul(
                          psq[:], lhsT=au[:, NQ + g:NQ + g + 1],
                          rhs=vpcs[:, goff + g, 0:16],
                          start=(g == 0), stop=(g == NQ - 1))

                  if stage == 4:
                      dbg = spool.tile([1, N_CLASSES], F32, tag="dbg4")
                      nc.vector.tensor_copy(dbg[:], psk[:, 0:N_CLASSES])
                      nc.sync.dma_start(out[b:b + 1, :], dbg[:])
                      continue

                  # ---- y = psk[0:5]/den_k + psq[5:10]/den_q + b ----
                  yk = spool.tile([1, N_CLASSES], F32, tag="yk")
                  nc.vector.tensor_scalar(
                      out=yk[:], in0=psk[:, 0:N_CLASSES],
                      scalar1=recip[:, 0:1], scalar2=None,
                      op0=mybir.AluOpType.mult)
                  yq = spool.tile([1, N_CLASSES], F32, tag="yq")
                  nc.vector.scalar_tensor_tensor(
                      out=yq[:], in0=psq[:, N_CLASSES:2 * N_CLASSES],
                      scalar=recip[:, 1:2], in1=yk[:],
                      op0=mybir.AluOpType.mult, op1=mybir.AluOpType.add)
                  y = spool.tile([1, N_CLASSES], F32, tag="y")
                  nc.vector.tensor_tensor(out=y[:], in0=yq[:], in1=linb_t[:],
                                          op=mybir.AluOpType.add)

                  # ---- log_softmax ----
                  ymax = spool.tile([1, 1], F32, tag="ymax")
                  nc.vector.reduce_max(ymax[:], y[:],
                                       axis=mybir.AxisListType.X)
                  tsb = spool.tile([1, N_CLASSES], F32, tag=f"tsb{b}")
                  nc.vector.tensor_scalar(
                      out=tsb[:], in0=y[:], scalar1=ymax[:], scalar2=None,
                      op0=mybir.AluOpType.subtract)
                  esb = spool.tile([1, N_CLASSES], F32, tag="esb")
                  nc.scalar.activation(esb[:], tsb[:],
                                       mybir.ActivationFunctionType.Exp,
                                       accum_out=sums_all[:, b:b + 1])
                  tsb_list.append(tsb)

              if stage > 4:
                  lsb = spool.tile([1, BPC], F32, tag="lsb")
                  nc.scalar.activation(lsb[:], sums_all[:],
                                       mybir.ActivationFunctionType.Ln)
                  for b, tsb in enumerate(tsb_list):
                      osb = spool.tile([1, N_CLASSES], F32, tag=f"osb{b}")
                      nc.vector.tensor_scalar(
                          out=osb[:], in0=tsb[:], scalar1=lsb[:, b:b + 1],
                          scalar2=None, op0=mybir.AluOpType.subtract)
                      nc.sync.dma_start(out[b:b + 1, :], osb[:])

    nc.compile()
    return nc


def prepare(inputs):
    """Host prep: returns (nc, in_maps, perm) for the 8-core SPMD launch."""
    return _prepare(**inputs)


def _prepare(token_seqs, pads, vec_table, covec_table, W, lin_w, lin_b):
    global _last_key
    token_seqs = np.asarray(token_seqs)
    pads = np.asarray(pads)
    vec_table = np.asarray(vec_table, dtype=np.float32)
    covec_table = np.asarray(covec_table, dtype=np.float32)
    W = np.asarray(W, dtype=np.float32)
    lin_w = np.asarray(lin_w, dtype=np.float32)
    lin_b = np.asarray(lin_b, dtype=np.float32)

    L = (S - pads).astype(np.int64)                      # [B] valid lengths

    # sort batches by L desc; slot j of core c takes rank 8*j + c
    perm = np.argsort(-L, kind="stable")
    slot_lens = tuple(
        int(np.ceil(L[perm[N_CORES * j]] / 128) * 128) for j in range(BPC)
    )

    # ---- vocab compaction (indices must fit int16 for dma_gather) ----
    uniq, inv = np.unique(token_seqs, return_inverse=True)
    inv = inv.reshape(B, S).astype(np.int64)
    U = len(uniq)
    zero_row = U                                          # all-zero pad row
    U1 = U + 1
    assert U1 <= 32768, "compact vocab must fit int16"

    vt_c = np.zeros((U1, D), np.float32)
    vt_c[:U] = vec_table[uniq]
    cvt_c = np.zeros((U1, D), np.float32)
    cvt_c[:U] = covec_table[uniq]

    # fused transposed-gather table: [vec@W | covec]
    vwcv_np = np.zeros((U1, 2 * D), np.float32)
    vwcv_np[:U, :D] = vt_c[:U] @ W
    vwcv_np[:, D:] = cvt_c
    vwcv_np = vwcv_np.astype(ml_dtypes.bfloat16)

    # classifier folded into per-token projections: [vp(5) | covp(5) | 0]
    # (rows padded to 128 elems = 256B -- dma_gather elem-size constraint)
    vpc_np = np.zeros((U1, 128), np.float32)
    vpc_np[:, 0:N_CLASSES] = vt_c @ lin_w[:, :D].T
    vpc_np[:, N_CLASSES:2 * N_CLASSES] = cvt_c @ lin_w[:, D:].T
    vpc_np = vpc_np.astype(ml_dtypes.bfloat16)

    # invalid positions -> zero row
    toks = inv.copy()
    pos = np.arange(S)[None, :]
    toks[pos >= L[:, None]] = zero_row
    toks = toks.astype(np.int16)

    linb_np = lin_b.reshape(1, N_CLASSES).astype(np.float32)

    key = (U1, slot_lens)
    _last_key = key
    if key not in _cache:
        _cache[key] = _build_program(U1, slot_lens)
    nc = _cache[key]

    NQS = [l // 128 for l in slot_lens]
    NI = int(sum(slot_lens))

    # ---- per-core staging ----
    in_maps = []
    for c in range(N_CORES):
        bsel = [int(perm[N_CORES * j + c]) for j in range(BPC)]
        tf = np.concatenate(
            [toks[b, :slot_lens[j]] for j, b in enumerate(bsel)])
        idx_np = np.zeros((16, NI // 16), np.int16)
        idx_np[np.arange(NI) % 16, np.arange(NI) // 16] = tf
        idx_np = np.tile(idx_np, (8, 1))

        # doubled masks: per slot, cols [2*noff : 2*noff+2*NQ] are
        # [k-mask(NQ) | q-mask(NQ)] (identical halves)
        msk_np = np.zeros((128, 2 * int(sum(NQS))), np.float32)
        col = 0
        for j, b in enumerate(bsel):
            half = np.zeros((128, NQS[j]), np.float32)
            for g in range(NQS[j]):
                s = g * 128 + np.arange(128)
                half[:, g] = np.where(s < L[b], 0.0, NEG)
            msk_np[:, col:col + NQS[j]] = half
            msk_np[:, col + NQS[j]:col + 2 * NQS[j]] = half
            col += 2 * NQS[j]

        in_maps.append({
            "vwcv": vwcv_np, "vpc": vpc_np,
            "idx": idx_np, "msk": msk_np, "linb": linb_np,
        })

    return nc, in_maps, perm


def kernel(token_seqs, pads, vec_table, covec_table, W, lin_w, lin_b):
    nc, in_maps, perm = _prepare(token_seqs, pads, vec_table, covec_table,
                                 W, lin_w, lin_b)
    res = run_bass_kernel_spmd(nc, in_maps, core_ids=list(range(N_CORES)))
    outs = np.zeros((B, N_CLASSES), np.float32)
    for c in range(N_CORES):
        o = res.results[c]["out"]
        for j in range(BPC):
            outs[perm[N_CORES * j + c]] = o[j]
    return outs


if __name__ == "__main__":
    import reference
    inputs = reference.setup_inputs()
    expected = np.asarray(reference.reference(**inputs))
    actual = kernel(**{k: np.asarray(v) for k, v in inputs.items()})
    err = np.abs(actual - expected).max()
    rel = np.linalg.norm(actual - expected) / np.linalg.norm(expected)
    print("max abs err:", err, "rel err:", rel)


# revision 15
# speedup vs baseline: 1.3353x; 1.3353x over previous
"""Trainium2 Bass kernel for nn_BasicAttention (ragged sequence attention).

Reference computation (per batch b, S=1024, D=256):
    vecs   = vec_table[tokens]          [S, D]
    covecs = covec_table[tokens]        [S, D]
    E      = (vecs @ W) @ covecs^T      [S, S]   (masked to valid prefix L_b)
    ak     = softmax(masked colmax(E)); aq = softmax(masked rowmax(E))
    out    = log_softmax(concat(ak@vecs, aq@covecs) @ lin_w^T + lin_b)

Strategy: data-parallel over batch (4 samples per core x 8 cores), samples
sorted by valid length L and distributed round-robin so each per-core
"slot" has a static extent (max L of its rank group, rounded to 128).

Design notes:
  - One fused transposed dma_gather per slot pulls [vecW | covec] rows in
    [d, s] layout (1KB rows); a small straight gather pulls per-token
    classifier projections [vec@lw1^T | covec@lw2^T] (256B rows) -- the
    classifier is folded into the embedding tables on the host, removing
    ~3MB/core of gather traffic vs gathering raw [vec|covec] rows.
  - PSUM eviction is fused with exp on ACT. Per-slot mode balances ACT vs
    DVE: 'lse' slots evict exp(T*(E-B0)) with accum_out = rowlse (an lse
    softening of the row max, T=64) and recover softmax weights via
    Ln/Exp; 'tree' slots evict exp(E) (T=1) and take the exact row max
    with a DVE pairwise-max tree (values are already exp(rowmax)).
  - Col max runs on exp values (monotone): DVE running-max chain over
    q-tiles, PE transpose, reduce.
  - A combined exp+ln ACT table (natural_log_exp_and_others) is loaded
    once up front so bacc never inserts per-function table switches.
  - Slots are emitted in software-pipelined order -- phase1(b) = matmuls +
    evictions + col chain, phase2(b) = reductions/softmax/classifier --
    as p1(0) p1(1) p2(0) p1(2) p2(1) ... so each engine's in-order queue
    always has the next slot's heavy work before a stalled tail.
  - Invalid positions use an all-zero table row, so E=0 there; with >=513
    valid entries the true max is > 0 w.h.p., so exp-of-zero entries never
    win the maxes and masks kill the rest.
"""

import numpy as np
import ml_dtypes

import concourse.bass as bass
import concourse.mybir as mybir
import concourse.tile as tile
from concourse import bacc
from concourse.bass_utils import run_bass_kernel_spmd
from concourse.masks import make_identity

# Problem constants (hardcoded per spec)
B = 32
S = 1024
D = 256
N_CLASSES = 5
N_CORES = 8
BPC = B // N_CORES          # batches (slots) per core
NEG = -30000.0              # large-negative mask (exp() underflows to 0)
T = 64.0                    # lse temperature
B0 = 1.2                    # exp range-safety bias: e_exp = exp(T*(E-B0))

BF16 = mybir.dt.bfloat16
F32 = mybir.dt.float32
I16 = mybir.dt.int16

_cache = {}
_last_key = None


def _tree_cycles(nq, kk):
    """DVE cycles for the pairwise-max tree over [128, nq, kk] bf16."""
    cyc, n = 0, kk
    while n > 16:
        cyc += 58 + (nq * n // 2) // 2
        n //= 2
    cyc += 58 + nq * n          # final 1x reduce
    return cyc


def _pick_modes(slot_lens):
    """Balance ACT vs DVE: bit j set -> slot j 'tree' mode, clear -> 'lse'.
    Cycle model: ACT@1.2GHz charges KK+172(init)+187(accum-read) per lse
    tile or KK+172 per tree tile; DVE@0.96 runs chains/reduces always and
    the pairwise tree for tree slots."""
    NQS = [l // 128 for l in slot_lens]
    pe_ns = (sum(2 * nq * kk for nq, kk in zip(NQS, slot_lens))
             + sum(NQS) * 134 + 2 * sum(NQS) * 60) / 2.4
    best = None
    for bits in range(2 ** len(slot_lens)):
        act, dve = 3000.0, 3000.0
        for j, (nq, kk) in enumerate(zip(NQS, slot_lens)):
            dve += (nq - 1) * (58 + kk // 2) + (58 + kk // 4) \
                + (120 + nq * 128)
            if bits >> j & 1:
                act += nq * (kk + 172) + 260
                dve += _tree_cycles(nq, kk)
            else:
                act += nq * (kk + 359) + 2 * 260
        t = max(act / 1.2, dve / 0.96, pe_ns)
        if best is None or t < best[0]:
            best = (t, bits)
    return best[1]


def _build_program(U1, slot_lens=(S,) * BPC, stage=99, repeat=1):
    """Per-core Bass program. U1 = compact table rows; slot_lens = static
    per-slot extents (multiples of 128, descending); repeat for benching."""
    import os
    stage = int(os.environ.get("K_STAGE", stage))
    nc = bacc.Bacc("TRN2", num_devices=N_CORES, debug=False)

    modes = _pick_modes(slot_lens)
    if os.environ.get("K_MODES"):
        modes = int(os.environ["K_MODES"])
    NI = int(sum(slot_lens))            # gathered tokens per core
    NQS = [l // 128 for l in slot_lens]  # per-slot q/k tile counts
    OFF = np.cumsum([0] + list(slot_lens))[:-1]      # token offsets
    NOFF = np.cumsum([0] + list(NQS))[:-1]           # mask-col offsets
    NMC = int(sum(NQS))                 # total mask cols (per half)

    # ---- DRAM I/O ----
    vwcv = nc.dram_tensor("vwcv", [U1, 2 * D], BF16, kind="ExternalInput").ap()
    vpc = nc.dram_tensor("vpc", [U1, 128], BF16, kind="ExternalInput").ap()
    idx = nc.dram_tensor("idx", [128, NI // 16], I16,
                         kind="ExternalInput").ap()
    msk = nc.dram_tensor("msk", [128, 2 * NMC], F32, kind="ExternalInput").ap()
    msk01 = nc.dram_tensor("msk01", [128, 2 * NMC], BF16,
                           kind="ExternalInput").ap()
    linb = nc.dram_tensor("linb", [1, N_CLASSES], F32,
                          kind="ExternalInput").ap()
    out = nc.dram_tensor("out", [BPC, N_CLASSES], F32,
                         kind="ExternalOutput").ap()

    with tile.TileContext(nc) as tc:
        with (
            tc.tile_pool(name="const", bufs=1) as cpool,
            tc.tile_pool(name="gath", bufs=2) as gpool,
            tc.tile_pool(name="eall", bufs=2) as epool,
            tc.tile_pool(name="accp", bufs=2) as apool,
            tc.tile_pool(name="tree", bufs=2) as tpool,
            tc.tile_pool(name="small", bufs=2) as spool,
            tc.tile_pool(name="ps_e", bufs=2, space="PSUM") as ps_e,
            tc.tile_pool(name="ps_t", bufs=2, space="PSUM") as ps_t,
            tc.tile_pool(name="ps_s", bufs=2, space="PSUM") as ps_s,
        ):
            # ---- constants / staging ----
            idx_t = cpool.tile([128, NI // 16], I16)
            nc.sync.dma_start(idx_t[:], idx)
            msk_t = cpool.tile([128, 2 * NMC], F32)
            nc.sync.dma_start(msk_t[:], msk)
            msk01_t = cpool.tile([128, 2 * NMC], BF16)
            nc.sync.dma_start(msk01_t[:], msk01)
            linb_t = cpool.tile([1, N_CLASSES], F32)
            nc.sync.dma_start(linb_t[:], linb)
            ident = cpool.tile([128, 128], BF16)
            make_identity(nc, ident[:])
            ones_b = cpool.tile([128, 1], BF16)
            nc.vector.memset(ones_b[:], 1.0)
            biasn_t = cpool.tile([128, 1], F32)
            nc.vector.memset(biasn_t[:], -T * B0)

            # Pre-place the combined exp+ln table so bacc's
            # insert_act_table_loads pass sees every Exp/Ln activation
            # already served and never thrashes between per-function sets.
            from concourse.hw_specs import get_activation_tables
            _tabs = list(get_activation_tables(nc.m.arch).keys())
            _set_id = _tabs.index("natural_log_exp_and_others")
            nc.scalar.add_instruction(
                mybir.InstLoadActFuncSet(
                    name=nc.get_next_instruction_name(),
                    act_func_set_id=_set_id, ins=[], outs=[]))

            for _rep in range(repeat):
              # ---- gathers ----
              # transposed [d,s] rows of [vecW | covec]: one gather per slot
              # so slot 0's matmuls start after one small gather
              vwcvTs = []
              for j in range(BPC):
                  KJ = int(slot_lens[j])
                  oj = int(OFF[j])
                  g_j = gpool.tile([128, 4, KJ], BF16, tag=f"vwcvT{j}")
                  nc.gpsimd.dma_gather(
                      out_ap=g_j[:], in_ap=vwcv,
                      idxs_ap=idx_t[:, oj // 16:(oj + KJ) // 16],
                      num_idxs=KJ, num_idxs_reg=KJ, elem_size=2 * D,
                      transpose=True, single_packet=False,
                  )
                  vwcvTs.append(g_j)
              # straight [s, 128] classifier-projection rows (cols 0:16 used)
              vpcs = gpool.tile([128, NI // 128, 128], BF16, tag="vpcs")
              nc.gpsimd.dma_gather(
                  out_ap=vpcs[:], in_ap=vpc, idxs_ap=idx_t[:],
                  num_idxs=NI, num_idxs_reg=NI, elem_size=128,
                  transpose=False, single_packet=False,
              )

              if stage == 0:
                  dbg = spool.tile([128, N_CLASSES], F32, tag="dbg")
                  nc.vector.tensor_copy(dbg[:], vpcs[:, 0, 0:N_CLASSES])
                  nc.sync.dma_start(out[:, :], dbg[0:BPC, :])

              # per-slot log_softmax staging (Ln deferred to one final
              # phase so ACT stays on one table set)
              sums_all = spool.tile([1, BPC], F32, tag="lsm")
              tsb_list = []
              slot_state = {}

              def phase1(b):
                  """E matmuls + exp evictions + col-max chain for slot b."""
                  NQ = NQS[b]
                  KK = int(slot_lens[b])
                  nkc = (KK + 511) // 512
                  tree_mode = bool(modes >> b & 1)
                  vw = vwcvTs[b]
                  e_all = epool.tile([128, NQ, KK], BF16, tag="eall")
                  acc = apool.tile([128, KK], BF16, tag="acc")
                  # lse2 cols [0:NQ] = col max of exp; [NQ:2NQ] = row
                  # lse (lse slots) / row max of exp (tree slots)
                  lse2 = spool.tile([128, 2 * NQ], F32, tag=f"lse2_{b}")
                  for qt in range(NQ):
                      pe = ps_e.tile([128, 1024], F32, tag="pe")
                      for h in range(2):
                          for kt in range(nkc):
                              kw = min(512, KK - kt * 512)
                              nc.tensor.matmul(
                                  pe[:, kt * 512:kt * 512 + kw],
                                  lhsT=vw[:, h, qt * 128:(qt + 1) * 128],
                                  rhs=vw[:, 2 + h, kt * 512:kt * 512 + kw],
                                  start=(h == 0), stop=(h == 1),
                              )
                      if tree_mode:
                          nc.scalar.activation(
                              e_all[:, qt, :], pe[:, 0:KK],
                              mybir.ActivationFunctionType.Exp)
                      else:
                          nc.scalar.activation(
                              e_all[:, qt, :], pe[:, 0:KK],
                              mybir.ActivationFunctionType.Exp,
                              scale=T, bias=biasn_t[:],
                              accum_out=lse2[:, NQ + qt:NQ + qt + 1])
                      if qt == 1:
                          nc.vector.tensor_tensor(
                              out=acc[:], in0=e_all[:, 0, :],
                              in1=e_all[:, 1, :], op=mybir.AluOpType.max)
                      elif qt > 1:
                          nc.vector.tensor_tensor(
                              out=acc[:], in0=acc[:], in1=e_all[:, qt, :],
                              op=mybir.AluOpType.max)
                  slot_state[b] = (e_all, acc, lse2)

              def phase2a(b):
                  """Row-max tree (tree slots) + col reduce: DVE/PE only."""
                  NQ = NQS[b]
                  KK = int(slot_lens[b])
                  tree_mode = bool(modes >> b & 1)
                  e_all, acc, lse2 = slot_state[b]

                  if tree_mode:
                      # ---- row max of exp(E): pairwise-max tree ----
                      ta = tpool.tile([128, NQ, KK // 2], BF16, tag="ta")
                      tb = tpool.tile([128, NQ, KK // 4], BF16, tag="tb")
                      ev = e_all[:]
                      n = KK
                      nc.vector.tensor_tensor(
                          out=ta[:, :, 0:n // 2], in0=ev[:, :, 0:n // 2],
                          in1=ev[:, :, n // 2:n], op=mybir.AluOpType.max)
                      cur, nxt, n = ta, tb, n // 2
                      while n > 16:
                          nc.vector.tensor_tensor(
                              out=nxt[:, :, 0:n // 2],
                              in0=cur[:, :, 0:n // 2],
                              in1=cur[:, :, n // 2:n],
                              op=mybir.AluOpType.max)
                          cur, nxt, n = nxt, cur, n // 2
                      nc.vector.reduce_max(lse2[:, NQ:2 * NQ],
                                           cur[:, :, 0:n],
                                           axis=mybir.AxisListType.X)

                  # ---- col max of exp values: PE transpose + reduce ----
                  pt = ps_t.tile([128, 1024], BF16, tag="pt")
                  for g in range(NQ):
                      nc.tensor.transpose(
                          pt[:, g * 128:(g + 1) * 128],
                          acc[:, g * 128:(g + 1) * 128], ident[:])
                  nc.vector.reduce_max(
                      lse2[:, 0:NQ],
                      pt[:, 0:NQ * 128].rearrange("p (g f) -> p g f", g=NQ),
                      axis=mybir.AxisListType.X)

              def phase2b(b):
                  """Softmax weights, weighted sums, classifier,
                  log_softmax staging: small ops, emitted after all the
                  heavy phases so no engine queue blocks on their deps."""
                  NQ = NQS[b]
                  KK = int(slot_lens[b])
                  off = int(OFF[b])
                  noff = int(NOFF[b])
                  tree_mode = bool(modes >> b & 1)
                  e_all, acc, lse2 = slot_state[b]

                  # ---- softmax numerators ----
                  au = spool.tile([128, 2 * NQ], BF16, tag="au")
                  if tree_mode:
                      # lse2 holds exp(col/row max); mask with 0/1
                      nc.vector.tensor_tensor(
                          out=au[:], in0=lse2[:],
                          in1=msk01_t[:, 2 * noff:2 * (noff + NQ)],
                          op=mybir.AluOpType.mult)
                  else:
                      # au = exp((ln(lse2)+msk)/T) = lse2^(1/T) masked
                      lnl = spool.tile([128, 2 * NQ], F32, tag="lnl")
                      nc.scalar.activation(lnl[:], lse2[:],
                                           mybir.ActivationFunctionType.Ln)
                      lnm = spool.tile([128, 2 * NQ], F32, tag="lnm")
                      nc.vector.tensor_tensor(
                          out=lnm[:], in0=lnl[:],
                          in1=msk_t[:, 2 * noff:2 * (noff + NQ)],
                          op=mybir.AluOpType.add)
                      nc.scalar.activation(au[:], lnm[:],
                                           mybir.ActivationFunctionType.Exp,
                                           scale=1.0 / T)

                  # ---- denominators ----
                  pden = ps_s.tile([1, 2 * NQ], F32, tag="ps_small")
                  nc.tensor.matmul(pden[:], lhsT=ones_b[:], rhs=au[:],
                                   start=True, stop=True)
                  den = spool.tile([1, 2], F32, tag="den")
                  nc.vector.reduce_sum(
                      den[:], pden[:].rearrange("p (h q) -> p h q", h=2),
                      axis=mybir.AxisListType.X)
                  recip = spool.tile([1, 2], F32, tag="recip")
                  nc.vector.reciprocal(recip[:], den[:])

                  # ---- weighted sums of classifier projections ----
                  # lhsT = au[:, [g, NQ+g]] -> row 0 = ak-weighted sums,
                  # row 1 = aq-weighted sums
                  psk = ps_s.tile([1, 16], F32, tag="ps_small")
                  psq = ps_s.tile([1, 16], F32, tag="ps_small")
                  goff = off // 128
                  for g in range(NQ):
                      nc.tensor.matmul(
                          psk[:], lhsT=au[:, g:g + 1],
                          rhs=vpcs[:, goff + g, 0:16],
                          start=(g == 0), stop=(g == NQ - 1))
                  for g in range(NQ):
                      nc.tensor.matmul(
                          psq[:], lhsT=au[:, NQ + g:NQ + g + 1],
                          rhs=vpcs[:, goff + g, 0:16],
                          start=(g == 0), stop=(g == NQ - 1))

                  # ---- y = psk[0:5]/den_k + psq[5:10]/den_q + b ----
                  yk = spool.tile([1, N_CLASSES], F32, tag="yk")
                  nc.vector.tensor_scalar(
                      out=yk[:], in0=psk[:, 0:N_CLASSES],
                      scalar1=recip[:, 0:1], scalar2=None,
                      op0=mybir.AluOpType.mult)
                  yq = spool.tile([1, N_CLASSES], F32, tag="yq")
                  nc.vector.scalar_tensor_tensor(
                      out=yq[:], in0=psq[:, N_CLASSES:2 * N_CLASSES],
                      scalar=recip[:, 1:2], in1=yk[:],
                      op0=mybir.AluOpType.mult, op1=mybir.AluOpType.add)
                  y = spool.tile([1, N_CLASSES], F32, tag="y")
                  nc.vector.tensor_tensor(out=y[:], in0=yq[:],
                                          in1=linb_t[:],
                                          op=mybir.AluOpType.add)

                  # ---- log_softmax staging ----
                  ymax = spool.tile([1, 1], F32, tag="ymax")
                  nc.vector.reduce_max(ymax[:], y[:],
                                       axis=mybir.AxisListType.X)
                  tsb = spool.tile([1, N_CLASSES], F32, tag=f"tsb{b}")
                  nc.vector.tensor_scalar(
                      out=tsb[:], in0=y[:], scalar1=ymax[:], scalar2=None,
                      op0=mybir.AluOpType.subtract)
                  esb = spool.tile([1, N_CLASSES], F32, tag="esb")
                  nc.scalar.activation(esb[:], tsb[:],
                                       mybir.ActivationFunctionType.Exp,
                                       accum_out=sums_all[:, b:b + 1])
                  tsb_list.append(tsb)

              if stage == 1:
                  for b in range(BPC):
                      phase1(b)
                      acc = slot_state[b][1]
                      dbg = spool.tile([128, N_CLASSES], F32, tag="dbg")
                      nc.vector.tensor_copy(dbg[:], acc[:, 0:N_CLASSES])
                      nc.sync.dma_start(out[b:b + 1, :], dbg[0:1, :])
              elif stage > 1:
                  # software-pipelined emission: every engine queue gets all
                  # heavy work (phase1 evictions/matmuls, phase2a
                  # trees/reduces) before any slot's small dependent tail
                  # (phase2b), so no head-of-line blocking
                  phase1(0)
                  phase1(1)
                  phase2a(0)
                  for b in range(2, BPC):
                      phase1(b)
                      phase2a(b - 1)
                  phase2a(BPC - 1)
                  for b in range(BPC):
                      phase2b(b)

              if stage > 1:
                  lsb = spool.tile([1, BPC], F32, tag="lsb")
                  nc.scalar.activation(lsb[:], sums_all[:],
                                       mybir.ActivationFunctionType.Ln)
                  for b, tsb in enumerate(tsb_list):
                      osb = spool.tile([1, N_CLASSES], F32, tag=f"osb{b}")
                      nc.vector.tensor_scalar(
                          out=osb[:], in0=tsb[:], scalar1=lsb[:, b:b + 1],
                          scalar2=None, op0=mybir.AluOpType.subtract)
                      nc.sync.dma_start(out[b:b + 1, :], osb[:])

    nc.compile()
    return nc


def prepare(inputs):
    """Host prep: returns (nc, in_maps, perm) for the 8-core SPMD launch."""
    return _prepare(**inputs)


def _prepare(token_seqs, pads, vec_table, covec_table, W, lin_w, lin_b):
    global _last_key
    token_seqs = np.asarray(token_seqs)
    pads = np.asarray(pads)
    vec_table = np.asarray(vec_table, dtype=np.float32)
    covec_table = np.asarray(covec_table, dtype=np.float32)
    W = np.asarray(W, dtype=np.float32)
    lin_w = np.asarray(lin_w, dtype=np.float32)
    lin_b = np.asarray(lin_b, dtype=np.float32)

    L = (S - pads).astype(np.int64)                      # [B] valid lengths

    # sort batches by L desc; slot j of core c takes rank 8*j + c
    perm = np.argsort(-L, kind="stable")
    slot_lens = tuple(
        int(np.ceil(L[perm[N_CORES * j]] / 128) * 128) for j in range(BPC)
    )

    # ---- vocab compaction (indices must fit int16 for dma_gather) ----
    uniq, inv = np.unique(token_seqs, return_inverse=True)
    inv = inv.reshape(B, S).astype(np.int64)
    U = len(uniq)
    zero_row = U                                          # all-zero pad row
    U1 = U + 1
    assert U1 <= 32768, "compact vocab must fit int16"

    vt_c = np.zeros((U1, D), np.float32)
    vt_c[:U] = vec_table[uniq]
    cvt_c = np.zeros((U1, D), np.float32)
    cvt_c[:U] = covec_table[uniq]

    # fused transposed-gather table: [vec@W | covec]
    vwcv_np = np.zeros((U1, 2 * D), np.float32)
    vwcv_np[:U, :D] = vt_c[:U] @ W
    vwcv_np[:, D:] = cvt_c
    vwcv_np = vwcv_np.astype(ml_dtypes.bfloat16)

    # classifier folded into per-token projections: [vp(5) | covp(5) | 0]
    # (rows padded to 128 elems = 256B -- dma_gather elem-size constraint)
    vpc_np = np.zeros((U1, 128), np.float32)
    vpc_np[:, 0:N_CLASSES] = vt_c @ lin_w[:, :D].T
    vpc_np[:, N_CLASSES:2 * N_CLASSES] = cvt_c @ lin_w[:, D:].T
    vpc_np = vpc_np.astype(ml_dtypes.bfloat16)

    # invalid positions -> zero row
    toks = inv.copy()
    pos = np.arange(S)[None, :]
    toks[pos >= L[:, None]] = zero_row
    toks = toks.astype(np.int16)

    linb_np = lin_b.reshape(1, N_CLASSES).astype(np.float32)

    key = (U1, slot_lens)
    _last_key = key
    if key not in _cache:
        _cache[key] = _build_program(U1, slot_lens)
    nc = _cache[key]

    NQS = [l // 128 for l in slot_lens]
    NI = int(sum(slot_lens))

    # ---- per-core staging ----
    in_maps = []
    for c in range(N_CORES):
        bsel = [int(perm[N_CORES * j + c]) for j in range(BPC)]
        tf = np.concatenate(
            [toks[b, :slot_lens[j]] for j, b in enumerate(bsel)])
        idx_np = np.zeros((16, NI // 16), np.int16)
        idx_np[np.arange(NI) % 16, np.arange(NI) // 16] = tf
        idx_np = np.tile(idx_np, (8, 1))

        # doubled masks: per slot, cols [2*noff : 2*noff+2*NQ] are
        # [k-mask(NQ) | q-mask(NQ)] (identical halves)
        msk_np = np.zeros((128, 2 * int(sum(NQS))), np.float32)
        col = 0
        for j, b in enumerate(bsel):
            half = np.zeros((128, NQS[j]), np.float32)
            for g in range(NQS[j]):
                s = g * 128 + np.arange(128)
                half[:, g] = np.where(s < L[b], 0.0, NEG)
            msk_np[:, col:col + NQS[j]] = half
            msk_np[:, col + NQS[j]:col + 2 * NQS[j]] = half
            col += 2 * NQS[j]

        in_maps.append({
            "vwcv": vwcv_np, "vpc": vpc_np, "idx": idx_np, "msk": msk_np,
            "msk01": (msk_np == 0.0).astype(ml_dtypes.bfloat16),
            "linb": linb_np,
        })

    return nc, in_maps, perm


def kernel(token_seqs, pads, vec_table, covec_table, W, lin_w, lin_b):
    nc, in_maps, perm = _prepare(token_seqs, pads, vec_table, covec_table,
                                 W, lin_w, lin_b)
    res = run_bass_kernel_spmd(nc, in_maps, core_ids=list(range(N_CORES)))
    outs = np.zeros((B, N_CLASSES), np.float32)
    for c in range(N_CORES):
        o = res.results[c]["out"]
        for j in range(BPC):
            outs[perm[N_CORES * j + c]] = o[j]
    return outs


if __name__ == "__main__":
    import reference
    inputs = reference.setup_inputs()
    expected = np.asarray(reference.reference(**inputs))
    actual = kernel(**{k: np.asarray(v) for k, v in inputs.items()})
    err = np.abs(actual - expected).max()
    rel = np.linalg.norm(actual - expected) / np.linalg.norm(expected)
    print("max abs err:", err, "rel err:", rel)


# revision 18
# speedup vs baseline: 1.5502x; 1.1609x over previous
"""Trainium2 Bass kernel for nn_BasicAttention (ragged sequence attention).

Reference computation (per batch b, S=1024, D=256):
    vecs   = vec_table[tokens]          [S, D]
    covecs = covec_table[tokens]        [S, D]
    E      = (vecs @ W) @ covecs^T      [S, S]   (masked to valid prefix L_b)
    ak     = softmax(masked colmax(E)); aq = softmax(masked rowmax(E))
    out    = log_softmax(concat(ak@vecs, aq@covecs) @ lin_w^T + lin_b)

Strategy: data-parallel over batch (4 samples per core x 8 cores), samples
sorted by valid length L and distributed round-robin so each per-core
"slot" has a static extent (max L of its rank group, rounded to 128).

Design notes:
  - One fused transposed dma_gather per slot pulls [vecW | covec] rows in
    [d, s] layout (1KB rows); a small straight gather pulls per-token
    classifier projections [vec@lw1^T | covec@lw2^T] (256B rows) -- the
    classifier is folded into the embedding tables on the host, removing
    ~3MB/core of gather traffic vs gathering raw [vec|covec] rows.
  - PSUM eviction is fused with exp on ACT. Per-slot mode balances ACT vs
    DVE: 'lse' slots evict exp(T*(E-B0)) with accum_out = rowlse (an lse
    softening of the row max, T=64) and recover softmax weights via
    Ln/Exp; 'tree' slots evict exp(E) (T=1) and take the exact row max
    with a DVE pairwise-max tree (values are already exp(rowmax)).
  - Col max runs on exp values (monotone): DVE running-max chain over
    q-tiles, PE transpose, reduce.
  - A combined exp+ln ACT table (natural_log_exp_and_others) is loaded
    once up front so bacc never inserts per-function table switches.
  - Slots are emitted in software-pipelined order -- phase1(b) = matmuls +
    evictions + col chain, phase2(b) = reductions/softmax/classifier --
    as p1(0) p1(1) p2(0) p1(2) p2(1) ... so each engine's in-order queue
    always has the next slot's heavy work before a stalled tail.
  - Invalid positions use an all-zero table row, so E=0 there; with >=513
    valid entries the true max is > 0 w.h.p., so exp-of-zero entries never
    win the maxes and masks kill the rest.
"""

import numpy as np
import ml_dtypes

import concourse.bass as bass
import concourse.mybir as mybir
import concourse.tile as tile
from concourse import bacc
from concourse.bass_utils import run_bass_kernel_spmd
from concourse.masks import make_identity

# Problem constants (hardcoded per spec)
B = 32
S = 1024
D = 256
N_CLASSES = 5
N_CORES = 8
BPC = B // N_CORES          # batches (slots) per core
NEG = -30000.0              # large-negative mask (exp() underflows to 0)
T = 64.0                    # lse temperature
B0 = 1.2                    # exp range-safety bias: e_exp = exp(T*(E-B0))

BF16 = mybir.dt.bfloat16
F32 = mybir.dt.float32
I16 = mybir.dt.int16

_cache = {}
_last_key = None


def _tree_cycles(nq, kk):
    """DVE cycles for the pairwise-max tree over [128, nq, kk] bf16."""
    cyc, n = 0, kk
    while n > 16:
        cyc += 58 + (nq * n // 2) // 2
        n //= 2
    cyc += 58 + nq * n          # final 1x reduce
    return cyc


def _pick_modes(slot_lens, kkes=None):
    """Balance ACT vs DVE: bit j set -> slot j 'tree' mode, clear -> 'lse'.
    Cycle model: ACT@1.2GHz charges KKe+172(init)+187(accum-read) per lse
    tile or KK+172 per tree tile (trees need full pow2-ish width); DVE@
    0.96 runs chains/reduces always and the pairwise tree for tree
    slots."""
    if kkes is None:
        kkes = slot_lens
    NQS = [l // 128 for l in slot_lens]
    pe_ns = (sum(2 * nq * kk for nq, kk in zip(NQS, slot_lens))
             + sum(NQS) * 134 + 2 * sum(NQS) * 60) / 2.4
    best = None
    for bits in range(2 ** len(slot_lens)):
        act, dve = 3000.0, 3000.0
        for j, (nq, kk) in enumerate(zip(NQS, slot_lens)):
            kke = kkes[j]
            dve += (120 + nq * 128)
            if bits >> j & 1:
                act += nq * (kk + 172) + 260
                dve += (nq - 1) * (58 + kk // 2) + _tree_cycles(nq, kk)
            else:
                act += nq * (kke + 359) + 2 * 260
                dve += (nq - 1) * (58 + kke // 2)
        t = max(act / 1.2, dve / 0.96, pe_ns)
        if best is None or t < best[0]:
            best = (t, bits)
    return best[1]


def _build_program(U1, slot_lens=(S,) * BPC, kkes=None, stage=99, repeat=1):
    """Per-core Bass program. U1 = compact table rows; slot_lens = static
    per-slot extents (multiples of 128, descending); repeat for benching."""
    import os
    stage = int(os.environ.get("K_STAGE", stage))
    nc = bacc.Bacc("TRN2", num_devices=N_CORES, debug=False)

    if kkes is None:
        kkes = slot_lens
    modes = _pick_modes(slot_lens, kkes)
    if os.environ.get("K_MODES"):
        modes = int(os.environ["K_MODES"])
    NI = int(sum(slot_lens))            # gathered tokens per core
    NQS = [l // 128 for l in slot_lens]  # per-slot q/k tile counts
    OFF = np.cumsum([0] + list(slot_lens))[:-1]      # token offsets
    NOFF = np.cumsum([0] + list(NQS))[:-1]           # mask-col offsets
    NMC = int(sum(NQS))                 # total mask cols (per half)

    # ---- DRAM I/O ----
    vwcv = nc.dram_tensor("vwcv", [U1, 2 * D], BF16, kind="ExternalInput").ap()
    vpc = nc.dram_tensor("vpc", [U1, 128], BF16, kind="ExternalInput").ap()
    idx = nc.dram_tensor("idx", [128, NI // 16], I16,
                         kind="ExternalInput").ap()
    msk = nc.dram_tensor("msk", [128, 2 * NMC], F32, kind="ExternalInput").ap()
    msk01 = nc.dram_tensor("msk01", [128, 2 * NMC], BF16,
                           kind="ExternalInput").ap()
    linb = nc.dram_tensor("linb", [1, N_CLASSES], F32,
                          kind="ExternalInput").ap()
    out = nc.dram_tensor("out", [BPC, N_CLASSES], F32,
                         kind="ExternalOutput").ap()

    with tile.TileContext(nc) as tc:
        with (
            tc.tile_pool(name="const", bufs=1) as cpool,
            tc.tile_pool(name="gath", bufs=2) as gpool,
            tc.tile_pool(name="eall", bufs=2) as epool,
            tc.tile_pool(name="accp", bufs=2) as apool,
            tc.tile_pool(name="tree", bufs=2) as tpool,
            tc.tile_pool(name="small", bufs=2) as spool,
            tc.tile_pool(name="ps_e", bufs=2, space="PSUM") as ps_e,
            tc.tile_pool(name="ps_t", bufs=2, space="PSUM") as ps_t,
            tc.tile_pool(name="ps_s", bufs=2, space="PSUM") as ps_s,
        ):
            # ---- constants / staging ----
            idx_t = cpool.tile([128, NI // 16], I16)
            nc.sync.dma_start(idx_t[:], idx)
            msk_t = cpool.tile([128, 2 * NMC], F32)
            nc.sync.dma_start(msk_t[:], msk)
            msk01_t = cpool.tile([128, 2 * NMC], BF16)
            nc.sync.dma_start(msk01_t[:], msk01)
            linb_t = cpool.tile([1, N_CLASSES], F32)
            nc.sync.dma_start(linb_t[:], linb)
            ident = cpool.tile([128, 128], BF16)
            make_identity(nc, ident[:])
            ones_b = cpool.tile([128, 1], BF16)
            nc.vector.memset(ones_b[:], 1.0)
            biasn_t = cpool.tile([128, 1], F32)
            nc.vector.memset(biasn_t[:], -T * B0)

            # Pre-place the combined exp+ln table so bacc's
            # insert_act_table_loads pass sees every Exp/Ln activation
            # already served and never thrashes between per-function sets.
            from concourse.hw_specs import get_activation_tables
            _tabs = list(get_activation_tables(nc.m.arch).keys())
            _set_id = _tabs.index("natural_log_exp_and_others")
            nc.scalar.add_instruction(
                mybir.InstLoadActFuncSet(
                    name=nc.get_next_instruction_name(),
                    act_func_set_id=_set_id, ins=[], outs=[]))

            for _rep in range(repeat):
              # ---- gathers ----
              # transposed [d,s] rows of [vecW | covec]: one gather per slot
              # so slot 0's matmuls start after one small gather
              vwcvTs = []
              for j in range(BPC):
                  KJ = int(slot_lens[j])
                  oj = int(OFF[j])
                  g_j = gpool.tile([128, 4, KJ], BF16, tag=f"vwcvT{j}")
                  nc.gpsimd.dma_gather(
                      out_ap=g_j[:], in_ap=vwcv,
                      idxs_ap=idx_t[:, oj // 16:(oj + KJ) // 16],
                      num_idxs=KJ, num_idxs_reg=KJ, elem_size=2 * D,
                      transpose=True, single_packet=False,
                  )
                  vwcvTs.append(g_j)
              # straight [s, 128] classifier-projection rows (cols 0:16 used)
              vpcs = gpool.tile([128, NI // 128, 128], BF16, tag="vpcs")
              nc.gpsimd.dma_gather(
                  out_ap=vpcs[:], in_ap=vpc, idxs_ap=idx_t[:],
                  num_idxs=NI, num_idxs_reg=NI, elem_size=128,
                  transpose=False, single_packet=False,
              )

              if stage == 0:
                  dbg = spool.tile([128, N_CLASSES], F32, tag="dbg")
                  nc.vector.tensor_copy(dbg[:], vpcs[:, 0, 0:N_CLASSES])
                  nc.sync.dma_start(out[:, :], dbg[0:BPC, :])

              # per-slot log_softmax staging (Ln deferred to one final
              # phase so ACT stays on one table set)
              sums_all = spool.tile([1, BPC], F32, tag="lsm")
              tsb_list = []
              slot_state = {}

              def phase1(b):
                  """E matmuls + exp evictions + col-max chain for slot b."""
                  NQ = NQS[b]
                  KK = int(slot_lens[b])
                  tree_mode = bool(modes >> b & 1)
                  # lse slots trim eviction/chain width to the true group
                  # max length (32-aligned); tree slots need full halving
                  # width
                  KKE = KK if tree_mode else int(kkes[b])
                  nkc = (KKE + 511) // 512
                  vw = vwcvTs[b]
                  e_all = epool.tile([128, NQ, KK], BF16, tag="eall")
                  acc = apool.tile([128, KK], BF16, tag="acc")
                  # lse2 cols [0:NQ] = col max of exp; [NQ:2NQ] = row
                  # lse (lse slots) / row max of exp (tree slots)
                  lse2 = spool.tile([128, 2 * NQ], F32, tag=f"lse2_{b}")
                  for qt in range(NQ):
                      pe = ps_e.tile([128, 1024], F32, tag="pe")
                      for h in range(2):
                          for kt in range(nkc):
                              kw = min(512, KKE - kt * 512)
                              nc.tensor.matmul(
                                  pe[:, kt * 512:kt * 512 + kw],
                                  lhsT=vw[:, h, qt * 128:(qt + 1) * 128],
                                  rhs=vw[:, 2 + h, kt * 512:kt * 512 + kw],
                                  start=(h == 0), stop=(h == 1),
                              )
                      if tree_mode:
                          nc.scalar.activation(
                              e_all[:, qt, 0:KKE], pe[:, 0:KKE],
                              mybir.ActivationFunctionType.Exp)
                      else:
                          nc.scalar.activation(
                              e_all[:, qt, 0:KKE], pe[:, 0:KKE],
                              mybir.ActivationFunctionType.Exp,
                              scale=T, bias=biasn_t[:],
                              accum_out=lse2[:, NQ + qt:NQ + qt + 1])
                      if qt == 0 and NQ == 1:
                          nc.vector.tensor_copy(acc[:, 0:KKE],
                                                e_all[:, 0, 0:KKE])
                      elif qt == 1:
                          nc.vector.tensor_tensor(
                              out=acc[:, 0:KKE], in0=e_all[:, 0, 0:KKE],
                              in1=e_all[:, 1, 0:KKE],
                              op=mybir.AluOpType.max)
                      elif qt > 1:
                          nc.vector.tensor_tensor(
                              out=acc[:, 0:KKE], in0=acc[:, 0:KKE],
                              in1=e_all[:, qt, 0:KKE],
                              op=mybir.AluOpType.max)
                  if KKE < KK:
                      # transpose/colreduce read full 128-blocks of acc
                      nc.vector.memset(acc[:, KKE:KK], 0.0)
                  slot_state[b] = (e_all, acc, lse2)

              def phase2a(b):
                  """Row-max tree (tree slots) + col reduce: DVE/PE only."""
                  NQ = NQS[b]
                  KK = int(slot_lens[b])
                  tree_mode = bool(modes >> b & 1)
                  e_all, acc, lse2 = slot_state[b]

                  if tree_mode:
                      # ---- row max of exp(E): pairwise-max tree ----
                      ta = tpool.tile([128, NQ, KK // 2], BF16, tag="ta")
                      tb = tpool.tile([128, NQ, KK // 4], BF16, tag="tb")
                      ev = e_all[:]
                      n = KK
                      nc.vector.tensor_tensor(
                          out=ta[:, :, 0:n // 2], in0=ev[:, :, 0:n // 2],
                          in1=ev[:, :, n // 2:n], op=mybir.AluOpType.max)
                      cur, nxt, n = ta, tb, n // 2
                      while n > 16:
                          nc.vector.tensor_tensor(
                              out=nxt[:, :, 0:n // 2],
                              in0=cur[:, :, 0:n // 2],
                              in1=cur[:, :, n // 2:n],
                              op=mybir.AluOpType.max)
                          cur, nxt, n = nxt, cur, n // 2
                      nc.vector.reduce_max(lse2[:, NQ:2 * NQ],
                                           cur[:, :, 0:n],
                                           axis=mybir.AxisListType.X)

                  # ---- col max of exp values: PE transpose + reduce ----
                  pt = ps_t.tile([128, 1024], BF16, tag="pt")
                  for g in range(NQ):
                      nc.tensor.transpose(
                          pt[:, g * 128:(g + 1) * 128],
                          acc[:, g * 128:(g + 1) * 128], ident[:])
                  nc.vector.reduce_max(
                      lse2[:, 0:NQ],
                      pt[:, 0:NQ * 128].rearrange("p (g f) -> p g f", g=NQ),
                      axis=mybir.AxisListType.X)

              def phase2b(b):
                  """Softmax weights, weighted sums, classifier,
                  log_softmax staging: small ops, emitted after all the
                  heavy phases so no engine queue blocks on their deps."""
                  NQ = NQS[b]
                  KK = int(slot_lens[b])
                  off = int(OFF[b])
                  noff = int(NOFF[b])
                  tree_mode = bool(modes >> b & 1)
                  e_all, acc, lse2 = slot_state[b]

                  # ---- softmax numerators ----
                  au = spool.tile([128, 2 * NQ], BF16, tag="au")
                  if tree_mode:
                      # lse2 holds exp(col/row max); mask with 0/1
                      nc.vector.tensor_tensor(
                          out=au[:], in0=lse2[:],
                          in1=msk01_t[:, 2 * noff:2 * (noff + NQ)],
                          op=mybir.AluOpType.mult)
                  else:
                      # au = exp((ln(lse2)+msk)/T) = lse2^(1/T) masked
                      lnl = spool.tile([128, 2 * NQ], F32, tag="lnl")
                      nc.scalar.activation(lnl[:], lse2[:],
                                           mybir.ActivationFunctionType.Ln)
                      lnm = spool.tile([128, 2 * NQ], F32, tag="lnm")
                      nc.vector.tensor_tensor(
                          out=lnm[:], in0=lnl[:],
                          in1=msk_t[:, 2 * noff:2 * (noff + NQ)],
                          op=mybir.AluOpType.add)
                      nc.scalar.activation(au[:], lnm[:],
                                           mybir.ActivationFunctionType.Exp,
                                           scale=1.0 / T)

                  # ---- denominators ----
                  pden = ps_s.tile([1, 2 * NQ], F32, tag="ps_small")
                  nc.tensor.matmul(pden[:], lhsT=ones_b[:], rhs=au[:],
                                   start=True, stop=True)
                  den = spool.tile([1, 2], F32, tag="den")
                  nc.vector.reduce_sum(
                      den[:], pden[:].rearrange("p (h q) -> p h q", h=2),
                      axis=mybir.AxisListType.X)
                  recip = spool.tile([1, 2], F32, tag="recip")
                  nc.vector.reciprocal(recip[:], den[:])

                  # ---- weighted sums of classifier projections ----
                  # lhsT = au[:, [g, NQ+g]] -> row 0 = ak-weighted sums,
                  # row 1 = aq-weighted sums
                  psk = ps_s.tile([1, 16], F32, tag="ps_small")
                  psq = ps_s.tile([1, 16], F32, tag="ps_small")
                  goff = off // 128
                  for g in range(NQ):
                      nc.tensor.matmul(
                          psk[:], lhsT=au[:, g:g + 1],
                          rhs=vpcs[:, goff + g, 0:16],
                          start=(g == 0), stop=(g == NQ - 1))
                  for g in range(NQ):
                      nc.tensor.matmul(
                          psq[:], lhsT=au[:, NQ + g:NQ + g + 1],
                          rhs=vpcs[:, goff + g, 0:16],
                          start=(g == 0), stop=(g == NQ - 1))

                  # ---- y = psk[0:5]/den_k + psq[5:10]/den_q + b ----
                  yk = spool.tile([1, N_CLASSES], F32, tag="yk")
                  nc.vector.tensor_scalar(
                      out=yk[:], in0=psk[:, 0:N_CLASSES],
                      scalar1=recip[:, 0:1], scalar2=None,
                      op0=mybir.AluOpType.mult)
                  yq = spool.tile([1, N_CLASSES], F32, tag="yq")
                  nc.vector.scalar_tensor_tensor(
                      out=yq[:], in0=psq[:, N_CLASSES:2 * N_CLASSES],
                      scalar=recip[:, 1:2], in1=yk[:],
                      op0=mybir.AluOpType.mult, op1=mybir.AluOpType.add)
                  y = spool.tile([1, N_CLASSES], F32, tag="y")
                  nc.vector.tensor_tensor(out=y[:], in0=yq[:],
                                          in1=linb_t[:],
                                          op=mybir.AluOpType.add)

                  # ---- log_softmax staging ----
                  ymax = spool.tile([1, 1], F32, tag="ymax")
                  nc.vector.reduce_max(ymax[:], y[:],
                                       axis=mybir.AxisListType.X)
                  tsb = spool.tile([1, N_CLASSES], F32, tag=f"tsb{b}")
                  nc.vector.tensor_scalar(
                      out=tsb[:], in0=y[:], scalar1=ymax[:], scalar2=None,
                      op0=mybir.AluOpType.subtract)
                  esb = spool.tile([1, N_CLASSES], F32, tag="esb")
                  nc.scalar.activation(esb[:], tsb[:],
                                       mybir.ActivationFunctionType.Exp,
                                       accum_out=sums_all[:, b:b + 1])
                  tsb_list.append(tsb)

              if stage == 1:
                  for b in range(BPC):
                      phase1(b)
                      acc = slot_state[b][1]
                      dbg = spool.tile([128, N_CLASSES], F32, tag="dbg")
                      nc.vector.tensor_copy(dbg[:], acc[:, 0:N_CLASSES])
                      nc.sync.dma_start(out[b:b + 1, :], dbg[0:1, :])
              elif stage > 1:
                  # software-pipelined emission: every engine queue gets all
                  # heavy work (phase1 evictions/matmuls, phase2a
                  # trees/reduces) before any slot's small dependent tail
                  # (phase2b), so no head-of-line blocking
                  phase1(0)
                  phase1(1)
                  phase2a(0)
                  for b in range(2, BPC):
                      phase1(b)
                      phase2a(b - 1)
                  phase2a(BPC - 1)
                  for b in range(BPC):
                      phase2b(b)

              if stage > 1:
                  lsb = spool.tile([1, BPC], F32, tag="lsb")
                  nc.scalar.activation(lsb[:], sums_all[:],
                                       mybir.ActivationFunctionType.Ln)
                  for b, tsb in enumerate(tsb_list):
                      osb = spool.tile([1, N_CLASSES], F32, tag=f"osb{b}")
                      nc.vector.tensor_scalar(
                          out=osb[:], in0=tsb[:], scalar1=lsb[:, b:b + 1],
                          scalar2=None, op0=mybir.AluOpType.subtract)
                      nc.sync.dma_start(out[b:b + 1, :], osb[:])

    nc.compile()
    return nc


def prepare(inputs):
    """Host prep: returns (nc, in_maps, perm) for the 8-core SPMD launch."""
    return _prepare(**inputs)


def _prepare(token_seqs, pads, vec_table, covec_table, W, lin_w, lin_b):
    global _last_key
    token_seqs = np.asarray(token_seqs)
    pads = np.asarray(pads)
    vec_table = np.asarray(vec_table, dtype=np.float32)
    covec_table = np.asarray(covec_table, dtype=np.float32)
    W = np.asarray(W, dtype=np.float32)
    lin_w = np.asarray(lin_w, dtype=np.float32)
    lin_b = np.asarray(lin_b, dtype=np.float32)

    L = (S - pads).astype(np.int64)                      # [B] valid lengths

    # sort batches by L desc; slot j of core c takes rank 8*j + c
    perm = np.argsort(-L, kind="stable")
    slot_lens = tuple(
        int(np.ceil(L[perm[N_CORES * j]] / 128) * 128) for j in range(BPC)
    )
    kkes = tuple(
        int(np.ceil(L[perm[N_CORES * j]] / 32) * 32) for j in range(BPC)
    )

    # ---- vocab compaction (indices must fit int16 for dma_gather) ----
    uniq, inv = np.unique(token_seqs, return_inverse=True)
    inv = inv.reshape(B, S).astype(np.int64)
    U = len(uniq)
    zero_row = U                                          # all-zero pad row
    U1 = U + 1
    assert U1 <= 32768, "compact vocab must fit int16"

    vt_c = np.zeros((U1, D), np.float32)
    vt_c[:U] = vec_table[uniq]
    cvt_c = np.zeros((U1, D), np.float32)
    cvt_c[:U] = covec_table[uniq]

    # fused transposed-gather table: [vec@W | covec]
    vwcv_np = np.zeros((U1, 2 * D), np.float32)
    vwcv_np[:U, :D] = vt_c[:U] @ W
    vwcv_np[:, D:] = cvt_c
    vwcv_np = vwcv_np.astype(ml_dtypes.bfloat16)

    # classifier folded into per-token projections: [vp(5) | covp(5) | 0]
    # (rows padded to 128 elems = 256B -- dma_gather elem-size constraint)
    vpc_np = np.zeros((U1, 128), np.float32)
    vpc_np[:, 0:N_CLASSES] = vt_c @ lin_w[:, :D].T
    vpc_np[:, N_CLASSES:2 * N_CLASSES] = cvt_c @ lin_w[:, D:].T
    vpc_np = vpc_np.astype(ml_dtypes.bfloat16)

    # invalid positions -> zero row
    toks = inv.copy()
    pos = np.arange(S)[None, :]
    toks[pos >= L[:, None]] = zero_row
    toks = toks.astype(np.int16)

    linb_np = lin_b.reshape(1, N_CLASSES).astype(np.float32)

    key = (U1, slot_lens, kkes)
    _last_key = key
    if key not in _cache:
        _cache[key] = _build_program(U1, slot_lens, kkes)
    nc = _cache[key]

    NQS = [l // 128 for l in slot_lens]
    NI = int(sum(slot_lens))

    # ---- per-core staging ----
    in_maps = []
    for c in range(N_CORES):
        bsel = [int(perm[N_CORES * j + c]) for j in range(BPC)]
        tf = np.concatenate(
            [toks[b, :slot_lens[j]] for j, b in enumerate(bsel)])
        idx_np = np.zeros((16, NI // 16), np.int16)
        idx_np[np.arange(NI) % 16, np.arange(NI) // 16] = tf
        idx_np = np.tile(idx_np, (8, 1))

        # doubled masks: per slot, cols [2*noff : 2*noff+2*NQ] are
        # [k-mask(NQ) | q-mask(NQ)] (identical halves)
        msk_np = np.zeros((128, 2 * int(sum(NQS))), np.float32)
        col = 0
        for j, b in enumerate(bsel):
            half = np.zeros((128, NQS[j]), np.float32)
            for g in range(NQS[j]):
                s = g * 128 + np.arange(128)
                half[:, g] = np.where(s < L[b], 0.0, NEG)
            msk_np[:, col:col + NQS[j]] = half
            msk_np[:, col + NQS[j]:col + 2 * NQS[j]] = half
            col += 2 * NQS[j]

        in_maps.append({
            "vwcv": vwcv_np, "vpc": vpc_np, "idx": idx_np, "msk": msk_np,
            "msk01": (msk_np == 0.0).astype(ml_dtypes.bfloat16),
            "linb": linb_np,
        })

    return nc, in_maps, perm


def kernel(token_seqs, pads, vec_table, covec_table, W, lin_w, lin_b):
    nc, in_maps, perm = _prepare(token_seqs, pads, vec_table, covec_table,
                                 W, lin_w, lin_b)
    res = run_bass_kernel_spmd(nc, in_maps, core_ids=list(range(N_CORES)))
    outs = np.zeros((B, N_CLASSES), np.float32)
    for c in range(N_CORES):
        o = res.results[c]["out"]
        for j in range(BPC):
            outs[perm[N_CORES * j + c]] = o[j]
    return outs


if __name__ == "__main__":
    import reference
    inputs = reference.setup_inputs()
    expected = np.asarray(reference.reference(**inputs))
    actual = kernel(**{k: np.asarray(v) for k, v in inputs.items()})
    err = np.abs(actual - expected).max()
    rel = np.linalg.norm(actual - expected) / np.linalg.norm(expected)
    print("max abs err:", err, "rel err:", rel)
